# revision 1
# baseline (speedup 1.0000x reference)
"""DoRA adapter forward kernel for 8 trn2 NeuronCores.

Math:  dora = dora_B @ dora_A                       [OUT, IN]
       num  = weight + ALPHA * dora                 [OUT, IN]
       s    = m / sqrt(colsum_over_out(num^2))      [1, IN]
       out  = x @ (num * s)^T + bias                (scale folded per-IN column)

Sharding (4x2 grid): 4-way split of the 8192 x-rows, 2-way split of OUT.
Each core:
  phase 1: streams its OUT-half of weight, computes num in bf16 (rank-16 dora
           via PE), squares (ACT) and column-sums via ones-matmul (PE) for the
           norm partial; stores num (bf16) to DRAM scratch; x is cast to bf16
           and round-tripped through DRAM so phase 2 can use the bf16-only
           DMA-transpose path. The sumsq partial [1, IN] is AllReduced across
           all 8 cores (each o-half is contributed by MG cores, so the sum is
           MG * full; folded into the sqrt's scale).
  phase 2: transposed loads of num^T and x^T; s = m/denom applied to the x^T
           stripes per-partition (out-of-place); dense bf16 GEMM (N=512,
           K-contiguous) + bias (pre-replicated via a K=1 ones-matmul), fp32
           accumulation in PSUM.

All HWDGE DMAs are issued on the single nc.sync (SP) ring on purpose: Tile
assigns HWDGE completion semaphores round-robin onto shared DMAHW lanes, but
hardware completion order is only FIFO per ring — mixing nc.sync and
nc.scalar DMAs in one kernel produces nondeterministic data races on trn2.
"""

import sys

if "/opt/trn_rl_repo" not in sys.path:
    sys.path.insert(0, "/opt/trn_rl_repo")

from contextlib import ExitStack

import numpy as np

import concourse.bass as bass
import concourse.mybir as mybir
import concourse.tile as tile
from concourse import bacc
from concourse.bass_utils import run_bass_kernel_spmd
from concourse.masks import make_identity
from concourse.tile_rust import add_dep_helper

F32 = mybir.dt.float32
BF16 = mybir.dt.bfloat16

ALPHA = 16.0
N_CORES = 8
MG, OG = 4, 2  # core grid: 4 m-groups x 2 o-halves

PHASE_BARRIER = False
OH_BARRIER = False
DBG = False
USE_CC = True  # 8-way AllReduce of sumsq partials instead of reading full W

# full problem sizes (hardcoded per contest contract)
B_, S_, IN_FULL, OUT_FULL, R_ = 4, 2048, 4096, 4096, 16
M_FULL = B_ * S_
M_C = M_FULL // MG      # 2048 x-rows per core
O_C = OUT_FULL // OG    # 2048 out-cols per core


def build_kernel(M_C, IN, O_C, O_OTH, R, n_cores=N_CORES):
    """Build the (core-agnostic SPMD) bass program."""
    IC = min(1024, IN)        # phase-1 i-chunk width
    MB = min(512, M_C)        # phase-2 m-block width
    OP = min(2048, O_C)       # phase-2 o-pass width
    assert IN % IC == 0 and M_C % MB == 0 and O_C % OP == 0
    assert IC % 512 == 0 and MB % 128 == 0 and OP % 512 == 0
    n_ic = IN // IC
    n_it = IN // 128          # i-tiles (contraction)

    nc = bacc.Bacc("TRN2", target_bir_lowering=False, debug=False,
                   num_devices=n_cores)

    x_in = nc.dram_tensor("x_slice", [M_C, IN], F32, kind="ExternalInput")
    w_own = nc.dram_tensor("w_own", [O_C, IN], F32, kind="ExternalInput")
    w_oth = b_oth = None
    if not USE_CC:
        w_oth = nc.dram_tensor("w_oth", [O_OTH, IN], F32, kind="ExternalInput")
    bias_in = nc.dram_tensor("bias_own", [1, O_C], F32, kind="ExternalInput")
    m_in = nc.dram_tensor("m_row", [1, IN], F32, kind="ExternalInput")
    a_in = nc.dram_tensor("dora_a", [R, IN], F32, kind="ExternalInput")
    b_own = nc.dram_tensor("dora_b_own", [O_C, R], F32, kind="ExternalInput")
    if not USE_CC:
        b_oth = nc.dram_tensor("dora_b_oth", [O_OTH, R], F32, kind="ExternalInput")
    out_t = nc.dram_tensor("out_slice", [M_C, O_C], F32, kind="ExternalOutput")

    # DRAM scratch
    x_bf = nc.dram_tensor("x_bf", [M_C, IN], BF16)
    num_bf = nc.dram_tensor("num_bf", [O_C, IN], BF16)
    s_dram = nc.dram_tensor("s_dram", [IN // 128, 128], F32)
    cc_out = None
    if USE_CC:
        cc_out = nc.dram_tensor("cc_out", [IN // 128, 128], F32,
                                addr_space="Shared")
    dbg_nt = dbg_numbf = None
    if DBG:
        dbg_nt = nc.dram_tensor(
            "dbg_nt", [(O_C // min(1024, O_C)) * IN, min(1024, O_C)], BF16,
            kind="ExternalOutput")
        dbg_numbf = nc.dram_tensor(
            "dbg_numbf", [O_C, IN], BF16, kind="ExternalOutput")

    with TileCtx(nc) as tc, ExitStack() as ctx:
        _emit(ctx, tc, locals())
    nc.compile()
    return nc


def TileCtx(nc):
    return tile.TileContext(nc)


def _emit(ctx, tc, v):
    nc = v["nc"]
    IN, R, IC, MB, OP = v["IN"], v["R"], v["IC"], v["MB"], v["OP"]
    M_C, O_C, O_OTH = v["M_C"], v["O_C"], v["O_OTH"]
    n_ic, n_it = v["n_ic"], v["n_it"]
    x_in, w_own, w_oth = v["x_in"], v["w_own"], v["w_oth"]
    bias_in, m_in, a_in = v["bias_in"], v["m_in"], v["a_in"]
    b_own, b_oth, out_t = v["b_own"], v["b_oth"], v["out_t"]
    x_bf, num_bf, s_dram = v["x_bf"], v["num_bf"], v["s_dram"]
    cc_out = v.get("cc_out")

    OUT_ALL = O_C if USE_CC else O_C + O_OTH

    # ---------------- constant / setup pools ----------------
    # `const` holds only what phase 2 reads (s_t, bias_rep, ones_row);
    # everything phase-1-only lives in `setup`, released before phase 2.
    const = ctx.enter_context(tc.tile_pool(name="const", bufs=1))
    setup_cm = tc.tile_pool(name="setup", bufs=1)
    setup = setup_cm.__enter__()

    ident = setup.tile([128, 128], F32, tag="ident")
    make_identity(nc, ident[:])

    ones_col = setup.tile([128, 1], BF16, tag="ones_col")
    nc.gpsimd.memset(ones_col[:], 1.0)
    ones_row = const.tile([1, 128], F32, tag="ones_row")
    nc.gpsimd.memset(ones_row[:], 1.0)

    # dora_A, cast to bf16 and pre-scaled by ALPHA (out-of-place)
    a_raw = setup.tile([R, IN], BF16, tag="a_raw")
    nc.gpsimd.dma_start(out=a_raw[:], in_=a_in[:, :])  # SWDGE cast f32->bf16
    a_bf = setup.tile([R, IN], BF16, tag="a_bf")
    nc.vector.tensor_scalar_mul(a_bf[:], a_raw[:], ALPHA)

    # dora_B transposed (both halves): BT[r, o] over o in [own | oth]
    bt_bf = setup.tile([R, OUT_ALL], BF16, tag="bt_bf")
    with tc.tile_pool(name="btmp", bufs=2) as btmp, \
         tc.tile_pool(name="setup_ps", bufs=2, space="PSUM") as setup_ps:
        bt_parts = (((b_own, O_C),) if USE_CC
                    else ((b_own, O_C), (b_oth, O_OTH)))
        for part, (b_src, o_sz) in enumerate(bt_parts):
            base = part * O_C
            for ot in range(o_sz // 128):
                b_t = btmp.tile([128, R], F32, tag="b_t")
                nc.sync.dma_start(out=b_t[:], in_=b_src[ot * 128:(ot + 1) * 128, :])
                ps = setup_ps.tile([R, 128], F32, tag="bt_ps")
                nc.tensor.transpose(ps[:], b_t[:], ident[:])
                nc.vector.tensor_copy(
                    out=bt_bf[:, base + ot * 128: base + (ot + 1) * 128], in_=ps[:])

    # ---------------- phase 1: num, colsum(num^2), x cast ----------------
    s_sb = setup.tile([1, IN], F32, tag="s_sb")  # colsum(num^2), then s

    if USE_CC:
        n_parts = ((w_own, b_own, O_C, True),)
    else:
        n_parts = ((w_own, b_own, O_C, True), (w_oth, b_oth, O_OTH, False))
    last_part_id = len(n_parts) - 1
    num_store_insts = {}
    x_store_insts = {}
    with tc.tile_pool(name="p1", bufs=3) as p1, \
         tc.tile_pool(name="xcast", bufs=3) as xcast, \
         tc.tile_pool(name="p1ps", bufs=2, space="PSUM") as p1ps, \
         tc.tile_pool(name="p1ps_s", bufs=1, space="PSUM") as p1ps_s:
        for ic in range(n_ic):
            c0 = ic * IC
            ps_s = p1ps_s.tile([1, IC], F32, tag="ps_s")
            first = True
            for part_id, (w_src, _b, o_sz, is_own) in enumerate(n_parts):
                base = 0 if is_own else O_C
                is_last_part = part_id == last_part_id
                for ot in range(o_sz // 128):
                    r0 = ot * 128
                    w_t = p1.tile([128, IC], F32, tag="w_t")
                    nc.sync.dma_start(out=w_t[:], in_=w_src[r0:r0 + 128, c0:c0 + IC])
                    ps_d = p1ps.tile([128, IC], F32, tag="ps_d")
                    for q in range(IC // 512):
                        nc.tensor.matmul(
                            ps_d[:, q * 512:(q + 1) * 512],
                            lhsT=bt_bf[:, base + r0: base + r0 + 128],
                            rhs=a_bf[:, c0 + q * 512: c0 + (q + 1) * 512],
                            start=True, stop=True)
                    num_t = p1.tile([128, IC], BF16, tag="num_t")
                    nc.vector.tensor_add(out=num_t[:], in0=w_t[:], in1=ps_d[:])
                    if is_own:
                        st = nc.gpsimd.dma_start(
                            out=num_bf[r0:r0 + 128, c0:c0 + IC], in_=num_t[:])
                        num_store_insts[(ic, ot)] = st.ins
                    sq_t = p1.tile([128, IC], BF16, tag="sq_t")
                    nc.scalar.square(sq_t[:], num_t[:])
                    last = is_last_part and (ot == o_sz // 128 - 1)
                    for q in range(IC // 512):
                        nc.tensor.matmul(
                            ps_s[0:1, q * 512:(q + 1) * 512],
                            lhsT=ones_col[:], rhs=sq_t[:, q * 512:(q + 1) * 512],
                            start=first, stop=last)
                    first = False
            nc.vector.tensor_copy(out=s_sb[0:1, c0:c0 + IC], in_=ps_s[0:1, :])

        # x -> bf16 round trip (independent of the norm)
        for mt in range(M_C // 128):
            xb = xcast.tile([128, IN], BF16, tag="xb")
            nc.gpsimd.dma_start(out=xb[:], in_=x_in[mt * 128:(mt + 1) * 128, :])
            st = nc.gpsimd.dma_start(
                out=x_bf[mt * 128:(mt + 1) * 128, :], in_=xb[:])
            x_store_insts[mt] = st.ins

    # ---- s = m / sqrt(colsum), in partition-major [128, IN//128] layout ----
    # roundtrip raw colsum through DRAM to redistribute across partitions
    # (all steps out-of-place)
    nc.gpsimd.dma_start(out=s_dram[:, :], in_=s_sb[0:1, :])
    s_raw = setup.tile([128, IN // 128], F32, tag="s_raw")
    if USE_CC:
        # each o-half partial is contributed by MG cores -> reduce = MG * full
        cc = nc.gpsimd.collective_compute(
            "AllReduce", mybir.AluOpType.add,
            ins=[s_dram.ap()], outs=[cc_out.ap()],
            replica_groups=[list(range(N_CORES))])
        ld = nc.sync.dma_start(
            out=s_raw[:], in_=cc_out.ap().rearrange("a b -> b a"))
        add_dep_helper(ld.ins, cc.ins, reason="s_raw RAW on collective out")
        sqrt_scale = 1.0 / MG
    else:
        nc.sync.dma_start(out=s_raw[:], in_=s_dram.ap().rearrange("a b -> b a"))
        sqrt_scale = 1.0
    s_sq = setup.tile([128, IN // 128], F32, tag="s_sq")
    nc.scalar.activation(s_sq[:], s_raw[:],
                         mybir.ActivationFunctionType.Sqrt, 0.0, sqrt_scale)
    s_rc = setup.tile([128, IN // 128], F32, tag="s_rc")
    nc.vector.reciprocal(s_rc[:], s_sq[:])
    m_t = setup.tile([128, IN // 128], F32, tag="m_t")
    nc.sync.dma_start(
        out=m_t[:], in_=m_in.ap().rearrange("a (c p) -> (a p) c", p=128))
    s_t = const.tile([128, IN // 128], F32, tag="s_t")
    nc.vector.tensor_mul(out=s_t[:], in0=s_rc[:], in1=m_t[:])

    # ---------------- bias replicated across partitions ----------------
    bias_sb = setup.tile([1, O_C], F32, tag="bias_sb")
    nc.sync.dma_start(out=bias_sb[0:1, :], in_=bias_in[:, :])
    bias_rep = const.tile([128, O_C], F32, tag="bias_rep")
    with tc.tile_pool(name="bias_ps", bufs=2, space="PSUM") as bias_ps:
        for oc in range(O_C // 512):
            ps_b = bias_ps.tile([128, 512], F32, tag="ps_b")
            nc.tensor.matmul(ps_b[:], lhsT=ones_row[:],
                             rhs=bias_sb[0:1, oc * 512:(oc + 1) * 512],
                             start=True, stop=True)
            nc.vector.tensor_copy(
                out=bias_rep[:, oc * 512:(oc + 1) * 512], in_=ps_b[:])

    setup_cm.__exit__(None, None, None)

    # ---------------- phase 2: out = (x * s) @ num^T + bias ----------------
    # numT/xT transposed loads feed PE directly; the per-i scale s is applied
    # to the x^T stripes out-of-place (DMA-written tiles are never rewritten).
    if PHASE_BARRIER:
        tc.strict_bb_all_engine_barrier()
    numT = ctx.enter_context(tc.tile_pool(name="numT", bufs=n_it))
    xT = ctx.enter_context(tc.tile_pool(name="xT", bufs=4))
    xS = ctx.enter_context(tc.tile_pool(name="xS", bufs=n_it + 8))
    p2ps = ctx.enter_context(tc.tile_pool(name="p2ps", bufs=2, space="PSUM"))
    outp = ctx.enter_context(tc.tile_pool(name="outp", bufs=3))

    for oh in range(O_C // OP):
        o0 = oh * OP
        if OH_BARRIER and oh > 0:
            tc.strict_bb_all_engine_barrier()
        nt_tiles = []
        for it in range(n_it):
            nt = numT.tile([128, OP], BF16, tag="nt")
            ld = nc.sync.dma_start_transpose(
                nt[:], num_bf[o0:o0 + OP, it * 128:(it + 1) * 128])
            ic = (it * 128) // IC
            for ot in range(o0 // 128, (o0 + OP) // 128):
                dep = num_store_insts.get((ic, ot))
                if dep is not None:
                    add_dep_helper(ld.ins, dep, reason="numT RAW on num_bf")
            if DBG and v.get("dbg_nt") is not None:
                nc.gpsimd.dma_start(
                    out=v["dbg_nt"][oh * IN + it * 128: oh * IN + (it + 1) * 128, :],
                    in_=nt[:])
            nt_tiles.append(nt)
        for mb in range(M_C // MB):
            m0 = mb * MB
            xs_tiles = []
            for it in range(n_it):
                xt = xT.tile([128, MB], BF16, tag="xt")
                ld = nc.sync.dma_start_transpose(
                    xt[:], x_bf[m0:m0 + MB, it * 128:(it + 1) * 128])
                for mt in range(m0 // 128, (m0 + MB) // 128):
                    dep = x_store_insts.get(mt)
                    if dep is not None:
                        add_dep_helper(ld.ins, dep, reason="xT RAW on x_bf")
                xs = xS.tile([128, MB], BF16, tag="xs")
                nc.vector.tensor_scalar_mul(xs[:], xt[:], s_t[:, it:it + 1])
                xs_tiles.append(xs)
            for mt in range(MB // 128):
                ps_o = p2ps.tile([128, OP], F32, tag="ps_o")
                for it in range(n_it):
                    lhsT = xs_tiles[it][:, mt * 128:(mt + 1) * 128]
                    for q in range(OP // 512):
                        nc.tensor.matmul(
                            ps_o[:, q * 512:(q + 1) * 512],
                            lhsT=lhsT,
                            rhs=nt_tiles[it][:, q * 512:(q + 1) * 512],
                            start=(it == 0), stop=(it == n_it - 1))
                OH = min(OP, 1024)
                for oseg in range(OP // OH):
                    s0 = oseg * OH
                    o_sb = outp.tile([128, OH], F32, tag="o_sb")
                    nc.vector.tensor_add(
                        out=o_sb[:], in0=ps_o[:, s0:s0 + OH],
                        in1=bias_rep[:, o0 + s0:o0 + s0 + OH])
                    nc.gpsimd.dma_start(
                        out=out_t[m0 + mt * 128: m0 + (mt + 1) * 128,
                                  o0 + s0:o0 + s0 + OH],
                        in_=o_sb[:])
    if DBG and v.get("dbg_numbf") is not None:
        nc.gpsimd.dma_start(out=v["dbg_numbf"][:, :], in_=num_bf[:, :])


_NC_CACHE = {}


def get_nc(M_C=M_C, IN=IN_FULL, O_C=O_C, O_OTH=OUT_FULL - O_C, R=R_):
    key = (M_C, IN, O_C, O_OTH, R)
    if key not in _NC_CACHE:
        _NC_CACHE[key] = build_kernel(M_C, IN, O_C, O_OTH, R)
    return _NC_CACHE[key]


def make_in_maps(x, weight, bias, m, dora_A, dora_B):
    x = np.ascontiguousarray(np.asarray(x, dtype=np.float32))
    weight = np.ascontiguousarray(np.asarray(weight, dtype=np.float32))
    bias = np.ascontiguousarray(np.asarray(bias, dtype=np.float32))
    m = np.ascontiguousarray(np.asarray(m, dtype=np.float32))
    dora_A = np.ascontiguousarray(np.asarray(dora_A, dtype=np.float32))
    dora_B = np.ascontiguousarray(np.asarray(dora_B, dtype=np.float32))
    xf = x.reshape(M_FULL, IN_FULL)
    in_maps = []
    for c in range(N_CORES):
        g, h = divmod(c, OG)
        o0 = h * O_C
        oo = (1 - h) * O_C
        im = {
            "x_slice": np.ascontiguousarray(xf[g * M_C:(g + 1) * M_C]),
            "w_own": np.ascontiguousarray(weight[o0:o0 + O_C]),
            "bias_own": np.ascontiguousarray(bias[o0:o0 + O_C].reshape(1, O_C)),
            "m_row": np.ascontiguousarray(m.reshape(1, IN_FULL)),
            "dora_a": dora_A,
            "dora_b_own": np.ascontiguousarray(dora_B[o0:o0 + O_C]),
        }
        if not USE_CC:
            im["w_oth"] = np.ascontiguousarray(weight[oo:oo + O_C])
            im["dora_b_oth"] = np.ascontiguousarray(dora_B[oo:oo + O_C])
        in_maps.append(im)
    return in_maps


def kernel(x, weight, bias, m, dora_A, dora_B, _trace=False, _trace_kwargs=None):
    in_maps = make_in_maps(x, weight, bias, m, dora_A, dora_B)
    res = run_bass_kernel_spmd(
        get_nc(), in_maps, core_ids=list(range(N_CORES)),
        trace=_trace, **(_trace_kwargs or {}))
    out = np.empty((M_FULL, OUT_FULL), np.float32)
    for c in range(N_CORES):
        g, h = divmod(c, OG)
        out[g * M_C:(g + 1) * M_C, h * O_C:(h + 1) * O_C] = \
            res.results[c]["out_slice"]
    ret = out.reshape(B_, S_, OUT_FULL)
    if _trace:
        return ret, res
    return ret

